# revision 33
# baseline (speedup 1.0000x reference)
"""Trainium2 Bass kernel for nn_Encoder_6339371729763.

6-layer shared-weight transformer encoder, B=4, S=2048, D=512, F=2048.
All 8 attention heads are identical -> attention is a single head with
HD=64 and tile(a, H) @ Wo collapses to a @ sum_of_Wo_blocks.

Sharding: 8 cores = 4 batch elements x 2 sequence halves. Each core owns
Sq=1024 query rows of one batch element. Per layer the pair of cores
sharing a batch element AllGathers k^T/v^T; attention over the local keys
runs while the AllGather is in flight.

v2 design notes (vs the 884us baseline):
- Residual adds moved off the DVE onto the PE: the FFN residual is an
  identity-matmul accumulated into the FFN2 PSUM; the attention residual
  uses diag(softmax_rowsum) @ x accumulated into the wo PSUM, which makes
  the 1/rowsum normalization commute with the residual.
- LayerNorm stats (bn_stats) read the PSUM accumulator directly; mean/
  rstd (and for LN1 the softmax reciprocal) are folded into a single ACT
  Identity(scale,bias) PSUM->SBUF evacuation. No y_sb intermediate, no
  scalar_tensor_tensor chain.
- Attention is processed per q-half so every PSUM tile is one bank:
  PSUM plan = 3 rotating 1-bank slots (transposes/qkv/scores/ffn1) +
  1 aT accumulator bank + 2x2-bank quarter accumulators (wo / ffn2).
- FFN per half = FFN1 over all f (hrelu stashed in SBUF), then FFN2 per
  2-row-tile quarter so LN2 pipelines behind the accumulation.
- ReLU evacuations alternate ACT/DVE; attT evacs on DVE, xt evacs on ACT.
- Warmup matmuls keep the PE HAM clock at 2.4GHz through the X DMA.
"""
import sys
import numpy as np

if "/opt/trn_rl_repo" not in sys.path:
    sys.path.insert(0, "/opt/trn_rl_repo")

import concourse.bass as bass
import concourse.tile as tile
from concourse import bacc, mybir
from concourse.bass_utils import run_bass_kernel_spmd
from concourse.masks import make_identity

F32 = mybir.dt.float32
F32R = mybir.dt.float32r
BF16 = mybir.dt.bfloat16
I32 = mybir.dt.int32
AF = mybir.ActivationFunctionType
ALU = mybir.AluOpType

B, S, D, H, F, L = 4, 2048, 512, 8, 2048, 6
HD = D // H          # 64
EPS = 1e-5
N_CORES = 8
SQ = S // 2          # 1024 rows per core
NT = SQ // 128       # 8 row tiles per core
KC = SQ // 128       # 8 key chunks per source (local / remote)
DC = D // 128        # 4
FC = F // 128        # 16
HT = NT // 2         # 4 row tiles per half

KV_T = BF16   # k/v AllGather transit dtype

_cache = {}


def _pos_encoding():
    pos = np.arange(S, dtype=np.float32).reshape(-1, 1)
    freqs = (0.0001 ** (2 * (np.arange(D, dtype=np.float32) // 2) / D)).reshape(1, -1)
    pe = pos * freqs
    pe[::2] = np.cos(pe[::2])
    pe[1::2] = np.sin(pe[1::2])
    return pe  # [S, D]


def _build():
    nc = bacc.Bacc(
        "TRN2",
        target_bir_lowering=False,
        debug=False,
        enable_asserts=True,
        num_devices=N_CORES,
    )
    X = nc.dram_tensor("X", [SQ, D], F32R, kind="ExternalInput").ap()
    Wqkv = nc.dram_tensor("Wqkv", [DC, 128, 3 * HD], BF16, kind="ExternalInput").ap()
    Wop = nc.dram_tensor("Wop", [HD, D], BF16, kind="ExternalInput").ap()
    Wf1 = nc.dram_tensor("Wf1", [DC, 128, F], BF16, kind="ExternalInput").ap()
    Wf2 = nc.dram_tensor("Wf2", [FC, 128, D], BF16, kind="ExternalInput").ap()
    OUT = nc.dram_tensor("OUT", [SQ, D], F32R, kind="ExternalOutput").ap()

    with tile.TileContext(nc) as tc:
        with (
            tc.tile_pool(name="wpool", bufs=1) as wp,
            tc.tile_pool(name="state", bufs=1) as st,
            tc.tile_pool(name="roll", bufs=3) as rl,
            tc.tile_pool(name="psR", bufs=3, space="PSUM") as psR,
            tc.tile_pool(name="psT", bufs=1, space="PSUM") as psT,
            tc.tile_pool(name="psQ", bufs=2, space="PSUM") as psQ,
            tc.tile_pool(name="dram", bufs=2, space="DRAM") as dram,
        ):
            # ---------------- constants / identities ----------------
            ident32 = wp.tile([128, 128], F32)
            make_identity(nc, ident32[:])
            identr = wp.tile([128, 128], F32R)
            nc.vector.tensor_copy(identr[:], ident32[:])
            # f32 identity at partitions 64-127 (for rowsum transposes)
            id64_32 = wp.tile([128, 64], F32)
            nc.vector.memset(id64_32[:], 0.0)
            nc.sync.dma_start(id64_32[64:128, :], ident32[0:64, 0:64])
            identr64 = wp.tile([128, 64], KV_T)
            nc.vector.tensor_copy(identr64[:], id64_32[:])

            wu_src = wp.tile([128, 512], KV_T)
            nc.vector.memset(wu_src[:], 0.0)

            # ---------------- inputs ----------------
            # X half 0 + qkv weights first (layer 0's critical path), the
            # big FFN weights stream on the ACT HWDGE queue in parallel.
            out_sb = st.tile([128, NT, D], F32R)  # residual stream
            Xr = X.rearrange("(t p) d -> p t d", p=128)
            nc.sync.dma_start(out_sb[:, 0:HT, :], Xr[:, 0:HT, :])
            wqkv_sb = wp.tile([128, DC, 3 * HD], KV_T)
            nc.sync.dma_start(wqkv_sb[:], Wqkv.rearrange("c p d -> p c d"))
            nc.sync.dma_start(out_sb[:, HT:NT, :], Xr[:, HT:NT, :])
            wf1_sb = wp.tile([128, DC, F], KV_T)
            nc.scalar.dma_start(wf1_sb[:], Wf1.rearrange("c p d -> p c d"))
            wf2_sb = wp.tile([128, FC, D], KV_T)
            nc.scalar.dma_start(wf2_sb[:], Wf2.rearrange("c p d -> p c d"))

            # PE warmup while the DMAs stream (keeps HAM at full clock)
            for w in range(20):
                wu_ps = psR.tile([128, 512], F32, tag="R", name=f"wu_{w}")
                nc.tensor.matmul(
                    wu_ps[:], wu_src[:, 0:128], wu_src[:], start=True, stop=True
                )

            wop_sb = wp.tile([128, D], KV_T)
            z32 = wp.tile([128, D], F32)
            nc.vector.memset(z32[:], 0.0)
            nc.vector.tensor_copy(wop_sb[:], z32[:])
            nc.sync.dma_start(wop_sb[0:HD, :], Wop[:])

            # v_aug: [keys 128, chunk, 128]; col HD all-ones (softmax denom),
            # cols HD+1.. stay zero so aT rows 65-127 are zeros
            v_aug = wp.tile([128, 2 * KC, 128], BF16)
            nc.vector.memset(v_aug[:], 0.0)
            ones32 = wp.tile([128, 2 * KC], F32)
            nc.vector.memset(ones32[:], 1.0)
            nc.vector.tensor_copy(v_aug[:, :, HD], ones32[:])

            # partner row offset in the flattened AllGather output
            pid = nc.partition_id(
                engines=[mybir.EngineType.Pool, mybir.EngineType.SP]
            )
            poff = (1 - (pid & 1)) * 128

            # ---------------- state tiles ----------------
            att_sb = st.tile([128, NT, D], F32R)
            xt_sb = st.tile([128, DC, SQ], KV_T)
            at_sb = st.tile([128, DC, SQ], KV_T)   # attT
            qt_sb = st.tile([128, SQ], KV_T)     # q^T at rows 0:64 AND 64:128
            kr_sb = st.tile([128, SQ], KV_T)       # partner kv (kT 0:64, vT 64:128)
            k2l_sb = st.tile([128, SQ], KV_T)      # local kT copy at rows 64:128
            k2r_sb = st.tile([128, SQ], KV_T)      # remote kT copy at rows 64:128
            aT_sb = st.tile([128, SQ], KV_T)
            rs_sb = st.tile([128, NT], F32)
            recip_sb = st.tile([128, NT], F32)
            r2_sb = st.tile([128, NT], F32)
            bnst = st.tile([128, NT, 6], F32)
            mv = st.tile([128, NT, 2], F32)
            vy = st.tile([128, NT], F32)
            nwt_t = st.tile([128, NT], F32)
            nwt_h = st.tile([128, NT], F32)
            rstd = st.tile([128, NT], F32)
            sc1 = st.tile([128, NT], F32)
            mr = st.tile([128, NT], F32)
            negb = st.tile([128, NT], F32)

            def newton_rsqrt(v_ap, out_ap, t_ap, h_ap, iters=2):
                """out = 1/sqrt(v), v > 0, on DVE."""
                nc.vector.tensor_scalar(
                    t_ap.bitcast(I32), v_ap.bitcast(I32), 1, None,
                    ALU.arith_shift_right,
                )
                nc.vector.tensor_scalar(
                    out_ap.bitcast(I32), t_ap.bitcast(I32), -1, 0x5F3759DF,
                    ALU.mult, op1=ALU.add,
                )
                for _ in range(iters):
                    nc.vector.tensor_mul(h_ap, out_ap, out_ap)
                    nc.vector.tensor_mul(h_ap, h_ap, v_ap)
                    nc.vector.tensor_scalar(h_ap, h_ap, -0.5, 1.5, ALU.mult, op1=ALU.add)
                    nc.vector.tensor_mul(out_ap, out_ap, h_ap)

            def transpose_half(src_tile, dst_tile, half, layer, nm):
                """dst[:, :, ft*128:(ft+1)*128] = src[:, ft, :]^T, ft in half.

                PSUM evacuations alternate ACT/DVE so neither engine
                serializes the chain."""
                for j in range(4):
                    ft = half * 4 + j
                    trp = psR.tile(
                        [128, 512], F32, tag="R",
                        name=f"{nm}_{layer}_{half}_{j}",
                    )
                    for pt in range(DC):
                        nc.tensor.transpose(
                            trp[:, pt * 128 : (pt + 1) * 128],
                            src_tile[:, ft, pt * 128 : (pt + 1) * 128].bitcast(F32),
                            ident32[:],
                        )
                    dst = dst_tile[:, :, ft * 128 : (ft + 1) * 128]
                    srcr = trp[:].rearrange("p (c n) -> p c n", c=DC)
                    if j % 2 == 0:
                        nc.scalar.activation(dst, srcr, AF.Copy)
                    else:
                        nc.vector.tensor_copy(dst, srcr)

            def vtp_build(src, soff, vbase, nch, layer, nm):
                """v_aug[:, vbase:vbase+nch, 0:HD] = chunk v^T from src rows 64:128."""
                vtp = psR.tile(
                    [128, nch * 64], KV_T, tag="R", name=f"vtp_{nm}_{layer}"
                )
                for j in range(nch):
                    nc.tensor.transpose(
                        vtp[:, j * 64 : (j + 1) * 64],
                        src[64:128, soff + j * 128 : soff + (j + 1) * 128],
                        identr64[64:128, :],
                    )
                nc.scalar.activation(
                    v_aug[:, vbase : vbase + nch, 0:HD], vtp[:], AF.Copy
                )

            # one attention state machine per q-half
            class AttnHalf:
                def __init__(self, half, layer):
                    self.half = half
                    self.layer = layer
                    self.n0 = half * 512
                    self.pend = []
                    self.flushed = 0
                    self.aT_ps = None

                def _flush(self):
                    e_sb, c = self.pend.pop(0)
                    nc.tensor.matmul(
                        self.aT_ps[:],
                        v_aug[:, c, :],
                        e_sb[:],
                        start=(self.flushed == 0),
                        stop=(self.flushed == 2 * KC - 1),
                    )
                    self.flushed += 1

                def drain(self, keep):
                    while len(self.pend) > keep:
                        self._flush()

                def pair(self, p, src, k2, vbase, defer=False):
                    """One even/odd chunk pair: 2 concurrent ETs, 2 exps, aT."""
                    if self.aT_ps is None:
                        self.aT_ps = psT.tile(
                            [128, 512], F32, tag="aT",
                            name=f"aT_{self.layer}_{self.half}",
                        )
                    h, ly, n0 = self.half, self.layer, self.n0
                    cA, cB = 2 * p, 2 * p + 1
                    etA = psR.tile(
                        [128, 512], F32, tag="R", name=f"etA_{ly}_{h}_{vbase}_{p}"
                    )
                    nc.tensor.matmul(
                        etA[:],
                        src[0:64, cA * 128 : (cA + 1) * 128],
                        qt_sb[0:64, n0 : n0 + 512],
                        start=True, stop=True,
                    )
                    etB = psR.tile(
                        [128, 512], F32, tag="R", name=f"etB_{ly}_{h}_{vbase}_{p}"
                    )
                    nc.tensor.matmul(
                        etB[:],
                        k2[64:128, cB * 128 : (cB + 1) * 128],
                        qt_sb[64:128, n0 : n0 + 512],
                        start=True, stop=True,
                    )
                    eA = rl.tile(
                        [128, 512], KV_T, tag="e", bufs=6,
                        name=f"eA_{ly}_{h}_{vbase}_{p}",
                    )
                    nc.scalar.activation(eA[:], etA[:], AF.Exp, scale=0.125)
                    self.pend.append((eA, vbase + cA))
                    eB = rl.tile(
                        [128, 512], KV_T, tag="e", bufs=6,
                        name=f"eB_{ly}_{h}_{vbase}_{p}",
                    )
                    nc.scalar.activation(eB[:], etB[:], AF.Exp, scale=0.125)
                    self.pend.append((eB, vbase + cB))
                    if not defer:
                        self.drain(2)

                def finish(self):
                    while self.pend:
                        self._flush()
                    # evacuate aT and extract rowsums / recip / recip^2
                    h, ly, n0 = self.half, self.layer, self.n0
                    t0, t1 = h * HT, (h + 1) * HT
                    nc.vector.tensor_copy(
                        aT_sb[:, n0 : n0 + 512], self.aT_ps[:]
                    )
                    rs_ps = psR.tile(
                        [128, HT, 2], KV_T, tag="R", name=f"rs_{ly}_{h}"
                    )
                    for i, t in enumerate(range(t0, t1)):
                        nc.tensor.transpose(
                            rs_ps[:, i, 0:1],
                            aT_sb[HD : HD + 1, t * 128 : (t + 1) * 128],
                            identr64[64:65, 0:1],
                        )
                    nc.vector.tensor_copy(rs_sb[:, t0:t1], rs_ps[:, :, 0])
                    nc.vector.reciprocal(recip_sb[:, t0:t1], rs_sb[:, t0:t1])
                    nc.vector.tensor_mul(
                        r2_sb[:, t0:t1], recip_sb[:, t0:t1], recip_sb[:, t0:t1]
                    )

            def wo_half_mm(half, q, layer):
                """wo PSUM accumulation for 2 row tiles: a@Wop + diag(rs)@x."""
                woq = psQ.tile(
                    [128, 2, D], F32, tag="Q", name=f"wo_{layer}_{half}_{q}"
                )
                for j in range(2):
                    t = half * HT + q * 2 + j
                    nc.tensor.matmul(
                        woq[:, j, :],
                        aT_sb[:, t * 128 : (t + 1) * 128],
                        wop_sb[:],
                        start=True, stop=False,
                    )
                    dg = rl.tile(
                        [128, 128], F32R, tag="diag", bufs=2,
                        name=f"dg_{layer}_{half}_{q}_{j}",
                    )
                    nc.vector.tensor_scalar(
                        dg[:], ident32[:], rs_sb[:, t : t + 1], None, ALU.mult
                    )
                    nc.tensor.matmul(
                        woq[:, j, :],
                        dg[:],
                        out_sb[:, t, :],
                        start=False, stop=True,
                    )
                for j in range(2):
                    t = half * HT + q * 2 + j
                    nc.vector.bn_stats(bnst[:, t, :], woq[:, j, :])
                    nc.vector.bn_aggr(mv[:, t, :], bnst[:, t, :])
                return woq

            def wo_ln(aps, t0, layer):
                """LN1 (folded softmax recip): one fixup chain over the
                per-tile PSUM APs, applies split across ACT and DVE."""
                sl = slice(t0, t0 + len(aps))
                nc.vector.tensor_mul(vy[:, sl], mv[:, sl, 1], r2_sb[:, sl])
                newton_rsqrt(
                    vy[:, sl], rstd[:, sl], nwt_t[:, sl], nwt_h[:, sl], iters=1
                )
                nc.vector.tensor_mul(sc1[:, sl], recip_sb[:, sl], rstd[:, sl])
                nc.vector.scalar_tensor_tensor(
                    negb[:, sl], mv[:, sl, 0], -1.0, sc1[:, sl],
                    op0=ALU.mult, op1=ALU.mult,
                )
                for j, ap in enumerate(aps):
                    t = t0 + j
                    if j % 2 == 0:
                        nc.scalar.activation(
                            att_sb[:, t, :], ap, AF.Identity,
                            bias=negb[:, t : t + 1], scale=sc1[:, t : t + 1],
                        )
                    else:
                        nc.vector.tensor_scalar(
                            att_sb[:, t, :], ap, sc1[:, t : t + 1],
                            negb[:, t : t + 1], ALU.mult, op1=ALU.add,
                        )

            def ffn1_half(half, layer):
                n0, n1 = half * 512, (half + 1) * 512
                hr = []
                for f in range(FC):
                    h_ps = psR.tile(
                        [128, 512], F32, tag="R", name=f"h_{layer}_{half}_{f}"
                    )
                    for c in range(DC):
                        nc.tensor.matmul(
                            h_ps[:],
                            wf1_sb[:, c, f * 128 : (f + 1) * 128],
                            at_sb[:, c, n0:n1],
                            start=(c == 0), stop=(c == DC - 1),
                        )
                    hrelu = rl.tile(
                        [128, 512], KV_T, tag="h", bufs=FC,
                        name=f"hr_{layer}_{half}_{f}",
                    )
                    if f % 2 == 0:
                        nc.scalar.activation(hrelu[:], h_ps[:], AF.Relu)
                    else:
                        nc.vector.tensor_scalar(
                            hrelu[:], h_ps[:], 0.0, None, ALU.max
                        )
                    hr.append(hrelu)
                return hr

            def ffn2_quarter_mm(hr, half, q, layer):
                """FFN2 accumulation (+x residual via identity mm) + stats."""
                ffq = psQ.tile(
                    [128, 2, D], F32, tag="Q", name=f"ffq_{layer}_{half}_{q}"
                )
                for j in range(2):
                    t = half * HT + q * 2 + j
                    nc.tensor.matmul(
                        ffq[:, j, :],
                        identr[:],
                        att_sb[:, t, :],
                        start=True, stop=False,
                    )
                for f in range(FC):
                    for j in range(2):
                        jj = q * 2 + j
                        nc.tensor.matmul(
                            ffq[:, j, :],
                            hr[f][:, jj * 128 : (jj + 1) * 128],
                            wf2_sb[:, f, :],
                            start=False, stop=(f == FC - 1),
                        )
                for j in range(2):
                    t = half * HT + q * 2 + j
                    nc.vector.bn_stats(bnst[:, t, :], ffq[:, j, :])
                    nc.vector.bn_aggr(mv[:, t, :], bnst[:, t, :])
                return ffq

            def ffn2_ln(ffs, half, q0, layer):
                """LN2: one fixup chain over the tiles of `ffs`, applies
                split across ACT and DVE."""
                t0 = half * HT + q0 * 2
                nq = len(ffs)
                sl = slice(t0, t0 + 2 * nq)
                nc.vector.tensor_copy(vy[:, sl], mv[:, sl, 1])
                newton_rsqrt(
                    vy[:, sl], rstd[:, sl], nwt_t[:, sl], nwt_h[:, sl], iters=1
                )
                nc.vector.scalar_tensor_tensor(
                    negb[:, sl], mv[:, sl, 0], -1.0, rstd[:, sl],
                    op0=ALU.mult, op1=ALU.mult,
                )
                for j in range(2 * nq):
                    t = t0 + j
                    ap = ffs[j // 2][:, j % 2, :]
                    if j % 2 == 0:
                        nc.scalar.activation(
                            out_sb[:, t, :], ap, AF.Identity,
                            bias=negb[:, t : t + 1], scale=rstd[:, t : t + 1],
                        )
                    else:
                        nc.vector.tensor_scalar(
                            out_sb[:, t, :], ap, rstd[:, t : t + 1],
                            negb[:, t : t + 1], ALU.mult, op1=ALU.add,
                        )
                    if layer == L - 1:
                        nc.sync.dma_start(
                            OUT.rearrange("(t p) d -> p t d", p=128)[:, t, :],
                            out_sb[:, t, :],
                        )

            def xt_qkv_half(lyr, half, kv_dst):
                """Project q/k/v for layer `lyr` half `half`; issue its AG."""
                transpose_half(out_sb, xt_sb, half, lyr, "xt")
                n0, n1 = half * 512, (half + 1) * 512
                kv_ps = psR.tile(
                    [128, 512], F32, tag="R", name=f"kvps_{lyr}_{half}"
                )
                for c in range(DC):
                    nc.tensor.matmul(
                        kv_ps[:],
                        wqkv_sb[:, c, 0:128],
                        xt_sb[:, c, n0:n1],
                        start=(c == 0), stop=(c == DC - 1),
                    )
                q_ps = psR.tile(
                    [64, 512], F32, tag="R", name=f"qps_{lyr}_{half}"
                )
                for c in range(DC):
                    nc.tensor.matmul(
                        q_ps[:],
                        wqkv_sb[:, c, 128:192],
                        xt_sb[:, c, n0:n1],
                        start=(c == 0), stop=(c == DC - 1),
                    )
                nc.scalar.activation(kv_dst[:, n0:n1], kv_ps[:], AF.Copy)
                nc.scalar.activation(qt_sb[0:64, n0:n1], q_ps[:], AF.Copy)
                nc.sync.dma_start(qt_sb[64:128, n0:n1], qt_sb[0:64, n0:n1])
                nc.sync.dma_start(k2l_sb[64:128, n0:n1], kv_dst[0:64, n0:n1])
                cc_in = dram.tile(
                    [128, 512], KV_T, tag=f"ccin{half}", name=f"ccin_{lyr}_{half}"
                )
                nc.sync.dma_start(cc_in[:], kv_dst[:, n0:n1])
                cc_out = dram.tile(
                    [256, 512], KV_T, tag=f"ccout{half}", name=f"ccout_{lyr}_{half}"
                )
                nc.gpsimd.collective_compute(
                    "AllGather",
                    ALU.bypass,
                    replica_groups=[[0, 1], [2, 3], [4, 5], [6, 7]],
                    ins=[cc_in.opt()],
                    outs=[cc_out.opt()],
                )
                return cc_out

            def fetch_remote_dma(cc_out, half):
                """Pull the partner's kv half out of the AG result."""
                n0, n1 = half * 512, (half + 1) * 512
                nc.sync.dma_start(
                    kr_sb[:, n0:n1], cc_out[bass.ds(poff, 128), :]
                )
                nc.sync.dma_start(k2r_sb[64:128, n0:n1], kr_sb[0:64, n0:n1])

            # ---------------- layer-0 prologue ----------------
            kv_cur = rl.tile([128, SQ], KV_T, tag="kvs", bufs=2, name="kvs_0")
            cc0 = xt_qkv_half(0, 0, kv_cur)
            vtp_build(kv_cur, 0, 0, 4, 0, "loc0")
            cc1 = xt_qkv_half(0, 1, kv_cur)
            vtp_build(kv_cur, 512, 4, 4, 0, "loc1")
            fetch_remote_dma(cc0, 0)
            fetch_remote_dma(cc1, 1)

            for layer in range(L):
                # ---------- attention half 0 (kv/qt/v_aug ready) ----------
                a0 = AttnHalf(0, layer)
                a1 = AttnHalf(1, layer)
                for p in range(KC // 2):
                    a0.pair(p, kv_cur, k2l_sb, 0)
                vtp_build(kr_sb, 0, KC, 4, layer, "rem0")
                a0.pair(0, kr_sb, k2r_sb, KC)
                a0.pair(1, kr_sb, k2r_sb, KC)
                vtp_build(kr_sb, 512, KC + 4, 4, layer, "rem1")
                a0.pair(2, kr_sb, k2r_sb, KC)
                a0.pair(3, kr_sb, k2r_sb, KC)
                a0.finish()

                # ---------- attention half 1, interleaved with wo(0) ------
                a1.pair(0, kv_cur, k2l_sb, 0)
                a1.pair(1, kv_cur, k2l_sb, 0)
                wo00 = wo_half_mm(0, 0, layer)
                a1.pair(2, kv_cur, k2l_sb, 0)
                a1.pair(3, kv_cur, k2l_sb, 0)
                wo01 = wo_half_mm(0, 1, layer)
                a1.pair(0, kr_sb, k2r_sb, KC)
                a1.pair(1, kr_sb, k2r_sb, KC)
                a1.pair(2, kr_sb, k2r_sb, KC)
                # the last remote pair + drain execute while the LN1(0)
                # fixup chain runs, keeping the PE warm through the seam
                wo_ln(
                    [wo00[:, 0, :], wo00[:, 1, :], wo01[:, 0, :], wo01[:, 1, :]],
                    0, layer,
                )
                a1.pair(3, kr_sb, k2r_sb, KC)
                a1.finish()
                transpose_half(att_sb, at_sb, 0, layer, "at")

                # ---------- FFN(0) with wo(1)/LN2(0) interleaved ----------
                hr0 = ffn1_half(0, layer)
                wo10 = wo_half_mm(1, 0, layer)
                wo_ln([wo10[:, 0, :], wo10[:, 1, :]], HT, layer)
                ff00 = ffn2_quarter_mm(hr0, 0, 0, layer)
                ffn2_ln([ff00], 0, 0, layer)
                wo11 = wo_half_mm(1, 1, layer)
                wo_ln([wo11[:, 0, :], wo11[:, 1, :]], HT + 2, layer)
                ff01 = ffn2_quarter_mm(hr0, 0, 1, layer)
                ffn2_ln([ff01], 0, 1, layer)
                transpose_half(att_sb, at_sb, 1, layer, "at")

                # ---------- FFN(1); next layer's projections + AllGathers
                # hide under these matmuls ----------
                if layer < L - 1:
                    kv_nxt = rl.tile(
                        [128, SQ], KV_T, tag="kvs", bufs=2,
                        name=f"kvs_{layer + 1}",
                    )
                    cc0 = xt_qkv_half(layer + 1, 0, kv_nxt)
                    vtp_build(kv_nxt, 0, 0, 4, layer + 1, "loc0")
                hr1 = ffn1_half(1, layer)
                ff10 = ffn2_quarter_mm(hr1, 1, 0, layer)
                ff11 = ffn2_quarter_mm(hr1, 1, 1, layer)
                ffn2_ln([ff10, ff11], 1, 0, layer)
                if layer < L - 1:
                    cc1 = xt_qkv_half(layer + 1, 1, kv_nxt)
                    vtp_build(kv_nxt, 512, 4, 4, layer + 1, "loc1")
                    fetch_remote_dma(cc0, 0)
                    fetch_remote_dma(cc1, 1)
                    kv_cur = kv_nxt

    nc.compile()
    return nc


def _prep_inputs(X, Wq, bq, Wk, bk, Wv, bv, Wo, bo, Wf1, bf1, Wf2, bf2,
                 ln1_g, ln1_b, ln2_g, ln2_b):
    f32 = np.float32
    for name, arr, want in [
        ("bq", bq, 0.0), ("bk", bk, 0.0), ("bv", bv, 0.0), ("bo", bo, 0.0),
        ("bf1", bf1, 0.0), ("bf2", bf2, 0.0),
        ("ln1_b", ln1_b, 0.0), ("ln2_b", ln2_b, 0.0),
        ("ln1_g", ln1_g, 1.0), ("ln2_g", ln2_g, 1.0),
    ]:
        assert np.allclose(np.asarray(arr), want, atol=0.0), (
            f"kernel specialized for trivial {name}"
        )
    import ml_dtypes
    bf16 = ml_dtypes.bfloat16
    X_pe = np.asarray(X, f32) + _pos_encoding()[None]  # [B, S, D]
    Wqkv = np.concatenate(
        [np.asarray(Wk, f32), np.asarray(Wv, f32), np.asarray(Wq, f32)], axis=1
    ).reshape(DC, 128, 3 * HD).astype(bf16)
    Wop = (
        np.asarray(Wo, f32).reshape(H, HD, D).astype(np.float64).sum(0)
        .astype(f32).astype(bf16)
    )
    Wf1r = np.asarray(Wf1, f32).reshape(DC, 128, F).astype(bf16)
    Wf2r = np.asarray(Wf2, f32).reshape(FC, 128, D).astype(bf16)
    in_maps = []
    for core in range(N_CORES):
        b, h = core // 2, core % 2
        in_maps.append({
            "X": np.ascontiguousarray(X_pe[b, h * SQ : (h + 1) * SQ]),
            "Wqkv": Wqkv, "Wop": Wop, "Wf1": Wf1r, "Wf2": Wf2r,
        })
    return in_maps


def _get_nc():
    if "nc" not in _cache:
        _cache["nc"] = _build()
    return _cache["nc"]


def kernel(**inputs) -> np.ndarray:
    nc = _get_nc()
    in_maps = _prep_inputs(**inputs)
    _cache["in_maps"] = in_maps
    res = run_bass_kernel_spmd(nc, in_maps, core_ids=list(range(N_CORES)))
    shards = [res.results[c]["OUT"] for c in range(N_CORES)]
    out = np.stack(shards).reshape(B, 2, SQ, D).reshape(B, S, D)
    return out


def profile_exec_time():
    """Re-run with NTFF tracing enabled; returns exec_time_ns (test.py use)."""
    import types
    import antenv
    import concourse.bass_utils as bu

    if "antenv.axon_hooks" not in sys.modules:
        mod = types.ModuleType("antenv.axon_hooks")
        _state = {"hook": None}
        mod.set_axon_ntff_profile_hook = lambda h: _state.__setitem__("hook", h)
        mod.get_axon_ntff_profile_hook = lambda: _state["hook"]
        sys.modules["antenv.axon_hooks"] = mod
        antenv.axon_hooks = mod
        from trn_agent_boot.trn_boot import _ntff_profile_via_ctypes
        mod.set_axon_ntff_profile_hook(
            _ntff_profile_via_ctypes("/opt/axon/libaxon_pjrt.so")
        )
        bu.upload_artifacts = lambda tmpdir: tmpdir
    nc = _get_nc()
    in_maps = _cache["in_maps"]
    res = run_bass_kernel_spmd(
        nc, in_maps, core_ids=list(range(N_CORES)), trace=True, trace_cores=[0]
    )
    _cache["last_trace"] = res.instructions_and_trace
    _cache["last_res"] = res
    return res.exec_time_ns


# revision 34
# speedup vs baseline: 1.1975x; 1.1975x over previous
"""Trainium2 Bass kernel for nn_Encoder_6339371729763.

6-layer shared-weight transformer encoder, B=4, S=2048, D=512, F=2048.
All 8 attention heads are identical -> attention is a single head with
HD=64 and tile(a, H) @ Wo collapses to a @ sum_of_Wo_blocks.

Sharding: 8 cores = 4 batch elements x 2 sequence halves. Each core owns
Sq=1024 query rows of one batch element. Per layer the pair of cores
sharing a batch element AllGathers k^T/v^T; attention over the local keys
runs while the AllGather is in flight.

v2 design notes (vs the 884us baseline):
- Residual adds moved off the DVE onto the PE: the FFN residual is an
  identity-matmul accumulated into the FFN2 PSUM; the attention residual
  uses diag(softmax_rowsum) @ x accumulated into the wo PSUM, which makes
  the 1/rowsum normalization commute with the residual.
- LayerNorm stats (bn_stats) read the PSUM accumulator directly; mean/
  rstd (and for LN1 the softmax reciprocal) are folded into a single ACT
  Identity(scale,bias) PSUM->SBUF evacuation. No y_sb intermediate, no
  scalar_tensor_tensor chain.
- Attention is processed per q-half so every PSUM tile is one bank:
  PSUM plan = 3 rotating 1-bank slots (transposes/qkv/scores/ffn1) +
  1 aT accumulator bank + 2x2-bank quarter accumulators (wo / ffn2).
- FFN per half = FFN1 over all f (hrelu stashed in SBUF), then FFN2 per
  2-row-tile quarter so LN2 pipelines behind the accumulation.
- ReLU evacuations alternate ACT/DVE; attT evacs on DVE, xt evacs on ACT.
- Warmup matmuls keep the PE HAM clock at 2.4GHz through the X DMA.
"""
import sys
import numpy as np

if "/opt/trn_rl_repo" not in sys.path:
    sys.path.insert(0, "/opt/trn_rl_repo")

import concourse.bass as bass
import concourse.tile as tile
from concourse import bacc, mybir
from concourse.bass_utils import run_bass_kernel_spmd
from concourse.masks import make_identity

F32 = mybir.dt.float32
F32R = mybir.dt.float32r
BF16 = mybir.dt.bfloat16
I32 = mybir.dt.int32
AF = mybir.ActivationFunctionType
ALU = mybir.AluOpType

B, S, D, H, F, L = 4, 2048, 512, 8, 2048, 6
HD = D // H          # 64
EPS = 1e-5
N_CORES = 8
SQ = S // 2          # 1024 rows per core
NT = SQ // 128       # 8 row tiles per core
KC = SQ // 128       # 8 key chunks per source (local / remote)
DC = D // 128        # 4
FC = F // 128        # 16
HT = NT // 2         # 4 row tiles per half

KV_T = BF16   # k/v AllGather transit dtype

_cache = {}


def _pos_encoding():
    pos = np.arange(S, dtype=np.float32).reshape(-1, 1)
    freqs = (0.0001 ** (2 * (np.arange(D, dtype=np.float32) // 2) / D)).reshape(1, -1)
    pe = pos * freqs
    pe[::2] = np.cos(pe[::2])
    pe[1::2] = np.sin(pe[1::2])
    return pe  # [S, D]


def _build():
    nc = bacc.Bacc(
        "TRN2",
        target_bir_lowering=False,
        debug=False,
        enable_asserts=True,
        num_devices=N_CORES,
    )
    X = nc.dram_tensor("X", [SQ, D], F32R, kind="ExternalInput").ap()
    Wqkv = nc.dram_tensor("Wqkv", [DC, 128, 3 * HD], BF16, kind="ExternalInput").ap()
    Wop = nc.dram_tensor("Wop", [HD, D], BF16, kind="ExternalInput").ap()
    Wf1 = nc.dram_tensor("Wf1", [DC, 128, F], BF16, kind="ExternalInput").ap()
    Wf2 = nc.dram_tensor("Wf2", [FC, 128, D], BF16, kind="ExternalInput").ap()
    OUT = nc.dram_tensor("OUT", [SQ, D], F32R, kind="ExternalOutput").ap()

    with tile.TileContext(nc) as tc:
        with (
            tc.tile_pool(name="wpool", bufs=1) as wp,
            tc.tile_pool(name="state", bufs=1) as st,
            tc.tile_pool(name="roll", bufs=3) as rl,
            tc.tile_pool(name="psR", bufs=3, space="PSUM") as psR,
            tc.tile_pool(name="psT", bufs=1, space="PSUM") as psT,
            tc.tile_pool(name="psQ", bufs=2, space="PSUM") as psQ,
            tc.tile_pool(name="dram", bufs=2, space="DRAM") as dram,
        ):
            # ---------------- constants / identities ----------------
            ident32 = wp.tile([128, 128], F32)
            make_identity(nc, ident32[:])
            identr = wp.tile([128, 128], F32R)
            nc.vector.tensor_copy(identr[:], ident32[:])
            # f32 identity at partitions 64-127 (for rowsum transposes)
            id64_32 = wp.tile([128, 64], F32)
            nc.vector.memset(id64_32[:], 0.0)
            nc.sync.dma_start(id64_32[64:128, :], ident32[0:64, 0:64])
            identr64 = wp.tile([128, 64], KV_T)
            nc.vector.tensor_copy(identr64[:], id64_32[:])

            wu_src = wp.tile([128, 512], KV_T)
            nc.vector.memset(wu_src[:], 0.0)

            # ---------------- inputs ----------------
            # X half 0 + qkv weights first (layer 0's critical path), the
            # big FFN weights stream on the ACT HWDGE queue in parallel.
            out_sb = st.tile([128, NT, D], F32R)  # residual stream
            Xr = X.rearrange("(t p) d -> p t d", p=128)
            nc.sync.dma_start(out_sb[:, 0:HT, :], Xr[:, 0:HT, :])
            wqkv_sb = wp.tile([128, DC, 3 * HD], KV_T)
            nc.sync.dma_start(wqkv_sb[:], Wqkv.rearrange("c p d -> p c d"))
            nc.sync.dma_start(out_sb[:, HT:NT, :], Xr[:, HT:NT, :])
            wf1_sb = wp.tile([128, DC, F], KV_T)
            nc.scalar.dma_start(wf1_sb[:], Wf1.rearrange("c p d -> p c d"))
            wf2_sb = wp.tile([128, FC, D], KV_T)
            nc.scalar.dma_start(wf2_sb[:], Wf2.rearrange("c p d -> p c d"))

            # PE warmup while the DMAs stream (keeps HAM at full clock)
            for w in range(20):
                wu_ps = psR.tile([128, 512], F32, tag="R", name=f"wu_{w}")
                nc.tensor.matmul(
                    wu_ps[:], wu_src[:, 0:128], wu_src[:], start=True, stop=True
                )

            wop_sb = wp.tile([128, D], KV_T)
            z32 = wp.tile([128, D], F32)
            nc.vector.memset(z32[:], 0.0)
            nc.vector.tensor_copy(wop_sb[:], z32[:])
            nc.sync.dma_start(wop_sb[0:HD, :], Wop[:])

            # v_aug: [keys 128, chunk, 128]; col HD all-ones (softmax denom),
            # cols HD+1.. stay zero so aT rows 65-127 are zeros
            v_aug = wp.tile([128, 2 * KC, 128], BF16)
            nc.vector.memset(v_aug[:], 0.0)
            ones32 = wp.tile([128, 2 * KC], F32)
            nc.vector.memset(ones32[:], 1.0)
            nc.vector.tensor_copy(v_aug[:, :, HD], ones32[:])

            # partner row offset in the flattened AllGather output
            pid = nc.partition_id(
                engines=[mybir.EngineType.Pool, mybir.EngineType.SP]
            )
            poff = (1 - (pid & 1)) * 128

            # ---------------- state tiles ----------------
            att_sb = st.tile([128, NT, D], F32R)
            xt_sb = st.tile([128, DC, SQ], KV_T)
            at_sb = st.tile([128, DC, SQ], KV_T)   # attT
            qt_sb = st.tile([128, SQ], KV_T)     # q^T at rows 0:64 AND 64:128
            kr_sb = st.tile([128, SQ], KV_T)       # partner kv (kT 0:64, vT 64:128)
            k2l_sb = st.tile([128, SQ], KV_T)      # local kT copy at rows 64:128
            k2r_sb = st.tile([128, SQ], KV_T)      # remote kT copy at rows 64:128
            aT_sb = st.tile([128, SQ], KV_T)
            rs_sb = st.tile([128, NT], F32)
            recip_sb = st.tile([128, NT], F32)
            r2_sb = st.tile([128, NT], F32)
            bnst = st.tile([128, NT, 6], F32)
            mv = st.tile([128, NT, 2], F32)
            vy = st.tile([128, NT], F32)
            nwt_t = st.tile([128, NT], F32)
            nwt_h = st.tile([128, NT], F32)
            rstd = st.tile([128, NT], F32)
            sc1 = st.tile([128, NT], F32)
            mr = st.tile([128, NT], F32)
            negb = st.tile([128, NT], F32)

            def newton_rsqrt(v_ap, out_ap, t_ap, h_ap, iters=2):
                """out = 1/sqrt(v), v > 0, on DVE."""
                nc.vector.tensor_scalar(
                    t_ap.bitcast(I32), v_ap.bitcast(I32), 1, None,
                    ALU.arith_shift_right,
                )
                nc.vector.tensor_scalar(
                    out_ap.bitcast(I32), t_ap.bitcast(I32), -1, 0x5F3759DF,
                    ALU.mult, op1=ALU.add,
                )
                for _ in range(iters):
                    nc.vector.tensor_mul(h_ap, out_ap, out_ap)
                    nc.vector.tensor_mul(h_ap, h_ap, v_ap)
                    nc.vector.tensor_scalar(h_ap, h_ap, -0.5, 1.5, ALU.mult, op1=ALU.add)
                    nc.vector.tensor_mul(out_ap, out_ap, h_ap)

            def transpose_half(src_tile, dst_tile, half, layer, nm):
                """dst[:, :, ft*128:(ft+1)*128] = src[:, ft, :]^T, ft in half.

                PSUM evacuations alternate ACT/DVE so neither engine
                serializes the chain."""
                for j in range(4):
                    ft = half * 4 + j
                    trp = psR.tile(
                        [128, 512], F32, tag="R",
                        name=f"{nm}_{layer}_{half}_{j}",
                    )
                    for pt in range(DC):
                        nc.tensor.transpose(
                            trp[:, pt * 128 : (pt + 1) * 128],
                            src_tile[:, ft, pt * 128 : (pt + 1) * 128].bitcast(F32),
                            ident32[:],
                        )
                    dst = dst_tile[:, :, ft * 128 : (ft + 1) * 128]
                    srcr = trp[:].rearrange("p (c n) -> p c n", c=DC)
                    if j % 2 == 0:
                        nc.scalar.activation(dst, srcr, AF.Copy)
                    else:
                        nc.vector.tensor_copy(dst, srcr)

            def vtp_build(src, soff, vbase, nch, layer, nm):
                """v_aug[:, vbase:vbase+nch, 0:HD] = chunk v^T from src rows 64:128."""
                vtp = psR.tile(
                    [128, nch * 64], KV_T, tag="R", name=f"vtp_{nm}_{layer}"
                )
                for j in range(nch):
                    nc.tensor.transpose(
                        vtp[:, j * 64 : (j + 1) * 64],
                        src[64:128, soff + j * 128 : soff + (j + 1) * 128],
                        identr64[64:128, :],
                    )
                nc.scalar.activation(
                    v_aug[:, vbase : vbase + nch, 0:HD], vtp[:], AF.Copy
                )

            # one attention state machine per q-half
            class AttnHalf:
                def __init__(self, half, layer):
                    self.half = half
                    self.layer = layer
                    self.n0 = half * 512
                    self.pend = []
                    self.flushed = 0
                    self.aT_ps = None

                def _flush(self):
                    e_sb, c = self.pend.pop(0)
                    nc.tensor.matmul(
                        self.aT_ps[:],
                        v_aug[:, c, :],
                        e_sb[:],
                        start=(self.flushed == 0),
                        stop=(self.flushed == 2 * KC - 1),
                    )
                    self.flushed += 1

                def drain(self, keep):
                    while len(self.pend) > keep:
                        self._flush()

                def pair(self, p, src, k2, vbase, defer=False):
                    """One even/odd chunk pair: 2 concurrent ETs, 2 exps, aT."""
                    if self.aT_ps is None:
                        self.aT_ps = psT.tile(
                            [128, 512], F32, tag="aT",
                            name=f"aT_{self.layer}_{self.half}",
                        )
                    h, ly, n0 = self.half, self.layer, self.n0
                    cA, cB = 2 * p, 2 * p + 1
                    etA = psR.tile(
                        [128, 512], F32, tag="R", name=f"etA_{ly}_{h}_{vbase}_{p}"
                    )
                    nc.tensor.matmul(
                        etA[:],
                        src[0:64, cA * 128 : (cA + 1) * 128],
                        qt_sb[0:64, n0 : n0 + 512],
                        start=True, stop=True,
                    )
                    etB = psR.tile(
                        [128, 512], F32, tag="R", name=f"etB_{ly}_{h}_{vbase}_{p}"
                    )
                    nc.tensor.matmul(
                        etB[:],
                        k2[64:128, cB * 128 : (cB + 1) * 128],
                        qt_sb[64:128, n0 : n0 + 512],
                        start=True, stop=True,
                    )
                    eA = rl.tile(
                        [128, 512], KV_T, tag="e", bufs=6,
                        name=f"eA_{ly}_{h}_{vbase}_{p}",
                    )
                    nc.scalar.activation(eA[:], etA[:], AF.Exp, scale=0.125)
                    self.pend.append((eA, vbase + cA))
                    eB = rl.tile(
                        [128, 512], KV_T, tag="e", bufs=6,
                        name=f"eB_{ly}_{h}_{vbase}_{p}",
                    )
                    nc.scalar.activation(eB[:], etB[:], AF.Exp, scale=0.125)
                    self.pend.append((eB, vbase + cB))
                    if not defer:
                        self.drain(2)

                def finish(self):
                    while self.pend:
                        self._flush()
                    # evacuate aT and extract rowsums / recip / recip^2
                    h, ly, n0 = self.half, self.layer, self.n0
                    t0, t1 = h * HT, (h + 1) * HT
                    nc.vector.tensor_copy(
                        aT_sb[:, n0 : n0 + 512], self.aT_ps[:]
                    )
                    rs_ps = psR.tile(
                        [128, HT, 2], KV_T, tag="R", name=f"rs_{ly}_{h}"
                    )
                    for i, t in enumerate(range(t0, t1)):
                        nc.tensor.transpose(
                            rs_ps[:, i, 0:1],
                            aT_sb[HD : HD + 1, t * 128 : (t + 1) * 128],
                            identr64[64:65, 0:1],
                        )
                    nc.vector.tensor_copy(rs_sb[:, t0:t1], rs_ps[:, :, 0])
                    nc.vector.reciprocal(recip_sb[:, t0:t1], rs_sb[:, t0:t1])
                    nc.vector.tensor_mul(
                        r2_sb[:, t0:t1], recip_sb[:, t0:t1], recip_sb[:, t0:t1]
                    )

            def wo_half_mm(half, q, layer):
                """wo PSUM accumulation for 2 row tiles: a@Wop + diag(rs)@x."""
                woq = psQ.tile(
                    [128, 2, D], F32, tag="Q", name=f"wo_{layer}_{half}_{q}"
                )
                for j in range(2):
                    t = half * HT + q * 2 + j
                    nc.tensor.matmul(
                        woq[:, j, :],
                        aT_sb[:, t * 128 : (t + 1) * 128],
                        wop_sb[:],
                        start=True, stop=False,
                    )
                    dg = rl.tile(
                        [128, 128], F32R, tag="diag", bufs=2,
                        name=f"dg_{layer}_{half}_{q}_{j}",
                    )
                    nc.vector.tensor_scalar(
                        dg[:], ident32[:], rs_sb[:, t : t + 1], None, ALU.mult
                    )
                    nc.tensor.matmul(
                        woq[:, j, :],
                        dg[:],
                        out_sb[:, t, :],
                        start=False, stop=True,
                    )
                for j in range(2):
                    t = half * HT + q * 2 + j
                    nc.vector.bn_stats(bnst[:, t, :], woq[:, j, :])
                    nc.vector.bn_aggr(mv[:, t, :], bnst[:, t, :])
                return woq

            def wo_ln(aps, t0, layer):
                """LN1 (folded softmax recip): one fixup chain over the
                per-tile PSUM APs, applies split across ACT and DVE."""
                sl = slice(t0, t0 + len(aps))
                nc.vector.tensor_mul(vy[:, sl], mv[:, sl, 1], r2_sb[:, sl])
                newton_rsqrt(
                    vy[:, sl], rstd[:, sl], nwt_t[:, sl], nwt_h[:, sl], iters=1
                )
                nc.vector.tensor_mul(sc1[:, sl], recip_sb[:, sl], rstd[:, sl])
                nc.vector.scalar_tensor_tensor(
                    negb[:, sl], mv[:, sl, 0], -1.0, sc1[:, sl],
                    op0=ALU.mult, op1=ALU.mult,
                )
                for j, ap in enumerate(aps):
                    t = t0 + j
                    if j % 2 == 0:
                        nc.scalar.activation(
                            att_sb[:, t, :], ap, AF.Identity,
                            bias=negb[:, t : t + 1], scale=sc1[:, t : t + 1],
                        )
                    else:
                        nc.vector.tensor_scalar(
                            att_sb[:, t, :], ap, sc1[:, t : t + 1],
                            negb[:, t : t + 1], ALU.mult, op1=ALU.add,
                        )

            def ffn1_half(half, layer):
                n0, n1 = half * 512, (half + 1) * 512
                hr = []
                for f in range(FC):
                    h_ps = psR.tile(
                        [128, 512], F32, tag="R", name=f"h_{layer}_{half}_{f}"
                    )
                    for c in range(DC):
                        nc.tensor.matmul(
                            h_ps[:],
                            wf1_sb[:, c, f * 128 : (f + 1) * 128],
                            at_sb[:, c, n0:n1],
                            start=(c == 0), stop=(c == DC - 1),
                        )
                    hrelu = rl.tile(
                        [128, 512], KV_T, tag="h", bufs=FC,
                        name=f"hr_{layer}_{half}_{f}",
                    )
                    if f % 2 == 0:
                        nc.scalar.activation(hrelu[:], h_ps[:], AF.Relu)
                    else:
                        nc.vector.tensor_scalar(
                            hrelu[:], h_ps[:], 0.0, None, ALU.max
                        )
                    hr.append(hrelu)
                return hr

            def ffn2_quarter_mm(hr, half, q, layer):
                """FFN2 accumulation (+x residual via identity mm) + stats."""
                ffq = psQ.tile(
                    [128, 2, D], F32, tag="Q", name=f"ffq_{layer}_{half}_{q}"
                )
                for j in range(2):
                    t = half * HT + q * 2 + j
                    nc.tensor.matmul(
                        ffq[:, j, :],
                        identr[:],
                        att_sb[:, t, :],
                        start=True, stop=False,
                    )
                for f in range(FC):
                    for j in range(2):
                        jj = q * 2 + j
                        nc.tensor.matmul(
                            ffq[:, j, :],
                            hr[f][:, jj * 128 : (jj + 1) * 128],
                            wf2_sb[:, f, :],
                            start=False, stop=(f == FC - 1),
                        )
                for j in range(2):
                    t = half * HT + q * 2 + j
                    nc.vector.bn_stats(bnst[:, t, :], ffq[:, j, :])
                    nc.vector.bn_aggr(mv[:, t, :], bnst[:, t, :])
                return ffq

            def ffn2_ln(ffs, half, q0, layer):
                """LN2: one fixup chain over the tiles of `ffs`, applies
                split across ACT and DVE."""
                t0 = half * HT + q0 * 2
                nq = len(ffs)
                sl = slice(t0, t0 + 2 * nq)
                nc.vector.tensor_copy(vy[:, sl], mv[:, sl, 1])
                newton_rsqrt(
                    vy[:, sl], rstd[:, sl], nwt_t[:, sl], nwt_h[:, sl], iters=1
                )
                nc.vector.scalar_tensor_tensor(
                    negb[:, sl], mv[:, sl, 0], -1.0, rstd[:, sl],
                    op0=ALU.mult, op1=ALU.mult,
                )
                for j in range(2 * nq):
                    t = t0 + j
                    ap = ffs[j // 2][:, j % 2, :]
                    if j % 2 == 0:
                        nc.scalar.activation(
                            out_sb[:, t, :], ap, AF.Identity,
                            bias=negb[:, t : t + 1], scale=rstd[:, t : t + 1],
                        )
                    else:
                        nc.vector.tensor_scalar(
                            out_sb[:, t, :], ap, rstd[:, t : t + 1],
                            negb[:, t : t + 1], ALU.mult, op1=ALU.add,
                        )
                    if layer == L - 1:
                        nc.sync.dma_start(
                            OUT.rearrange("(t p) d -> p t d", p=128)[:, t, :],
                            out_sb[:, t, :],
                        )

            def xt_qkv_half(lyr, half, kv_dst):
                """Project q/k/v for layer `lyr` half `half`; issue its AG."""
                transpose_half(out_sb, xt_sb, half, lyr, "xt")
                n0, n1 = half * 512, (half + 1) * 512
                kv_ps = psR.tile(
                    [128, 512], F32, tag="R", name=f"kvps_{lyr}_{half}"
                )
                for c in range(DC):
                    nc.tensor.matmul(
                        kv_ps[:],
                        wqkv_sb[:, c, 0:128],
                        xt_sb[:, c, n0:n1],
                        start=(c == 0), stop=(c == DC - 1),
                    )
                q_ps = psR.tile(
                    [64, 512], F32, tag="R", name=f"qps_{lyr}_{half}"
                )
                for c in range(DC):
                    nc.tensor.matmul(
                        q_ps[:],
                        wqkv_sb[:, c, 128:192],
                        xt_sb[:, c, n0:n1],
                        start=(c == 0), stop=(c == DC - 1),
                    )
                nc.scalar.activation(kv_dst[:, n0:n1], kv_ps[:], AF.Copy)
                nc.scalar.activation(qt_sb[0:64, n0:n1], q_ps[:], AF.Copy)
                nc.sync.dma_start(qt_sb[64:128, n0:n1], qt_sb[0:64, n0:n1])
                nc.sync.dma_start(k2l_sb[64:128, n0:n1], kv_dst[0:64, n0:n1])
                cc_in = dram.tile(
                    [128, 512], KV_T, tag=f"ccin{half}", name=f"ccin_{lyr}_{half}"
                )
                nc.sync.dma_start(cc_in[:], kv_dst[:, n0:n1])
                cc_out = dram.tile(
                    [256, 512], KV_T, tag=f"ccout{half}", name=f"ccout_{lyr}_{half}"
                )
                nc.gpsimd.collective_compute(
                    "AllGather",
                    ALU.bypass,
                    replica_groups=[[0, 1], [2, 3], [4, 5], [6, 7]],
                    ins=[cc_in.opt()],
                    outs=[cc_out.opt()],
                )
                return cc_out

            def fetch_remote_dma(cc_out, half):
                """Pull the partner's kv half out of the AG result."""
                n0, n1 = half * 512, (half + 1) * 512
                nc.sync.dma_start(
                    kr_sb[:, n0:n1], cc_out[bass.ds(poff, 128), :]
                )
                nc.sync.dma_start(k2r_sb[64:128, n0:n1], kr_sb[0:64, n0:n1])

            # ---------------- layer-0 prologue ----------------
            kv_cur = rl.tile([128, SQ], KV_T, tag="kvs", bufs=2, name="kvs_0")
            cc0 = xt_qkv_half(0, 0, kv_cur)
            vtp_build(kv_cur, 0, 0, 4, 0, "loc0")
            cc1 = xt_qkv_half(0, 1, kv_cur)
            vtp_build(kv_cur, 512, 4, 4, 0, "loc1")
            fetch_remote_dma(cc0, 0)
            fetch_remote_dma(cc1, 1)

            for layer in range(L):
                # ---------- attention half 0 (kv/qt/v_aug ready) ----------
                a0 = AttnHalf(0, layer)
                a1 = AttnHalf(1, layer)
                for p in range(KC // 2):
                    a0.pair(p, kv_cur, k2l_sb, 0)
                vtp_build(kr_sb, 0, KC, 4, layer, "rem0")
                a0.pair(0, kr_sb, k2r_sb, KC)
                a0.pair(1, kr_sb, k2r_sb, KC)
                vtp_build(kr_sb, 512, KC + 4, 4, layer, "rem1")
                a0.pair(2, kr_sb, k2r_sb, KC)
                a0.pair(3, kr_sb, k2r_sb, KC)
                a0.finish()

                # ---------- attention half 1, interleaved with wo(0) ------
                a1.pair(0, kv_cur, k2l_sb, 0)
                a1.pair(1, kv_cur, k2l_sb, 0)
                wo00 = wo_half_mm(0, 0, layer)
                a1.pair(2, kv_cur, k2l_sb, 0)
                a1.pair(3, kv_cur, k2l_sb, 0)
                wo01 = wo_half_mm(0, 1, layer)
                a1.pair(0, kr_sb, k2r_sb, KC)
                a1.pair(1, kr_sb, k2r_sb, KC)
                a1.pair(2, kr_sb, k2r_sb, KC)
                a1.pair(3, kr_sb, k2r_sb, KC)
                a1.finish()
                wo_ln(
                    [wo00[:, 0, :], wo00[:, 1, :], wo01[:, 0, :], wo01[:, 1, :]],
                    0, layer,
                )
                transpose_half(att_sb, at_sb, 0, layer, "at")

                # ---------- FFN(0) with wo(1)/LN2(0) interleaved ----------
                hr0 = ffn1_half(0, layer)
                wo10 = wo_half_mm(1, 0, layer)
                wo_ln([wo10[:, 0, :], wo10[:, 1, :]], HT, layer)
                ff00 = ffn2_quarter_mm(hr0, 0, 0, layer)
                ffn2_ln([ff00], 0, 0, layer)
                wo11 = wo_half_mm(1, 1, layer)
                wo_ln([wo11[:, 0, :], wo11[:, 1, :]], HT + 2, layer)
                ff01 = ffn2_quarter_mm(hr0, 0, 1, layer)
                ffn2_ln([ff01], 0, 1, layer)
                transpose_half(att_sb, at_sb, 1, layer, "at")

                # ---------- FFN(1); next layer's projections + AllGathers
                # hide under these matmuls ----------
                if layer < L - 1:
                    kv_nxt = rl.tile(
                        [128, SQ], KV_T, tag="kvs", bufs=2,
                        name=f"kvs_{layer + 1}",
                    )
                    cc0 = xt_qkv_half(layer + 1, 0, kv_nxt)
                    vtp_build(kv_nxt, 0, 0, 4, layer + 1, "loc0")
                hr1 = ffn1_half(1, layer)
                ff10 = ffn2_quarter_mm(hr1, 1, 0, layer)
                ff11 = ffn2_quarter_mm(hr1, 1, 1, layer)
                ffn2_ln([ff10, ff11], 1, 0, layer)
                if layer < L - 1:
                    cc1 = xt_qkv_half(layer + 1, 1, kv_nxt)
                    vtp_build(kv_nxt, 512, 4, 4, layer + 1, "loc1")
                    fetch_remote_dma(cc0, 0)
                    fetch_remote_dma(cc1, 1)
                    kv_cur = kv_nxt

    nc.compile()
    return nc


def _prep_inputs(X, Wq, bq, Wk, bk, Wv, bv, Wo, bo, Wf1, bf1, Wf2, bf2,
                 ln1_g, ln1_b, ln2_g, ln2_b):
    f32 = np.float32
    for name, arr, want in [
        ("bq", bq, 0.0), ("bk", bk, 0.0), ("bv", bv, 0.0), ("bo", bo, 0.0),
        ("bf1", bf1, 0.0), ("bf2", bf2, 0.0),
        ("ln1_b", ln1_b, 0.0), ("ln2_b", ln2_b, 0.0),
        ("ln1_g", ln1_g, 1.0), ("ln2_g", ln2_g, 1.0),
    ]:
        assert np.allclose(np.asarray(arr), want, atol=0.0), (
            f"kernel specialized for trivial {name}"
        )
    import ml_dtypes
    bf16 = ml_dtypes.bfloat16
    X_pe = np.asarray(X, f32) + _pos_encoding()[None]  # [B, S, D]
    Wqkv = np.concatenate(
        [np.asarray(Wk, f32), np.asarray(Wv, f32), np.asarray(Wq, f32)], axis=1
    ).reshape(DC, 128, 3 * HD).astype(bf16)
    Wop = (
        np.asarray(Wo, f32).reshape(H, HD, D).astype(np.float64).sum(0)
        .astype(f32).astype(bf16)
    )
    Wf1r = np.asarray(Wf1, f32).reshape(DC, 128, F).astype(bf16)
    Wf2r = np.asarray(Wf2, f32).reshape(FC, 128, D).astype(bf16)
    in_maps = []
    for core in range(N_CORES):
        b, h = core // 2, core % 2
        in_maps.append({
            "X": np.ascontiguousarray(X_pe[b, h * SQ : (h + 1) * SQ]),
            "Wqkv": Wqkv, "Wop": Wop, "Wf1": Wf1r, "Wf2": Wf2r,
        })
    return in_maps


def _get_nc():
    if "nc" not in _cache:
        _cache["nc"] = _build()
    return _cache["nc"]


def kernel(**inputs) -> np.ndarray:
    nc = _get_nc()
    in_maps = _prep_inputs(**inputs)
    _cache["in_maps"] = in_maps
    res = run_bass_kernel_spmd(nc, in_maps, core_ids=list(range(N_CORES)))
    shards = [res.results[c]["OUT"] for c in range(N_CORES)]
    out = np.stack(shards).reshape(B, 2, SQ, D).reshape(B, S, D)
    return out


def profile_exec_time():
    """Re-run with NTFF tracing enabled; returns exec_time_ns (test.py use)."""
    import types
    import antenv
    import concourse.bass_utils as bu

    if "antenv.axon_hooks" not in sys.modules:
        mod = types.ModuleType("antenv.axon_hooks")
        _state = {"hook": None}
        mod.set_axon_ntff_profile_hook = lambda h: _state.__setitem__("hook", h)
        mod.get_axon_ntff_profile_hook = lambda: _state["hook"]
        sys.modules["antenv.axon_hooks"] = mod
        antenv.axon_hooks = mod
        from trn_agent_boot.trn_boot import _ntff_profile_via_ctypes
        mod.set_axon_ntff_profile_hook(
            _ntff_profile_via_ctypes("/opt/axon/libaxon_pjrt.so")
        )
        bu.upload_artifacts = lambda tmpdir: tmpdir
    nc = _get_nc()
    in_maps = _cache["in_maps"]
    res = run_bass_kernel_spmd(
        nc, in_maps, core_ids=list(range(N_CORES)), trace=True, trace_cores=[0]
    )
    _cache["last_trace"] = res.instructions_and_trace
    _cache["last_res"] = res
    return res.exec_time_ns
